# revision 4
# baseline (speedup 1.0000x reference)
"""Kimi-style MoE (8 routed experts top-2 + shared expert) on 8 Trainium2 cores.

Sparse expert-parallel design. Core c owns routed expert c and computes it only
for the tokens actually routed to it (top-2 of 8 => ~T/4 tokens, capacity
C=768), plus a 1/8 intermediate-dim shard of the shared expert.

On-device dispatch per core:
  gate (fp32, replicated) -> top-2 mask -> my-expert mask -> val = token_id|-1
  -> DRAM fold to token order -> [16, T/16] wrapped + sentinel block
  -> gpsimd.sparse_gather (hardware stream compaction) -> compacted token ids
  -> clamp/cast int16 -> replicate to 128 partitions
  -> gpsimd.dma_gather(transpose=True): gathers x rows from HBM transposed
     into the matmul layout [dsub, ko, c].

Each core returns: compacted routed expert outputs [D, C] (no gating applied),
the compacted token ids, the per-token gate weights, and a dense [D, T] shared
partial. The host applies gating + biases and scatter-adds (the baseline
already did its 8-way partial reduce on host).

Expert matmuls run in bf16 (fp32 PSUM); the gate runs in fp32 because top-k
selection is precision-critical.
"""

import sys

for _p in ("/opt/trn_rl_repo", "/opt/pypackages"):
    if _p not in sys.path:
        sys.path.insert(0, _p)

import numpy as np
import ml_dtypes

import concourse.bass as bass
import concourse.mybir as mybir
import concourse.tile as tile
from concourse import bacc
from concourse.bass import ts
from concourse.bass_utils import run_bass_kernel_spmd

BF16 = mybir.dt.bfloat16
F32 = mybir.dt.float32
I16 = mybir.dt.int16
U32 = mybir.dt.uint32
NP_BF16 = ml_dtypes.bfloat16

# Problem shapes (hardcoded per the contract).
B, S, D = 2, 1024, 1024
E, TOPK = 8, 2
I = 1408
N_SHARED = 2
I_SH = N_SHARED * I          # 2816
SCALE = 2.5
T = B * S                    # 2048
P = 128
NT = T // 512                # 4 free-dim tiles of 512 tokens (shared expert)
KO = D // P                  # 8 contraction subtiles
JR = I // P                  # 11 routed (v,g) pair tiles
JS = 3                       # shared pair tiles per core (padded)
KD = JR + JS                 # 14 down-proj contraction tiles
DT = D // P                  # 8 output partition tiles
MT = T // P                  # 16 token tiles in the gate
N_CORES = 8

C = 768                      # routed gather capacity (dma_gather needs %128)
C16 = C // 16                # 48
CC = 704                     # routed compute capacity (>= max expert count)
CT = 2                       # token tiles for routed compute
CTS = CC // CT               # 352

BIG = 1.0e9


def _body(tc, io, uid=0):
    nc = tc.nc
    add = mybir.AluOpType.add
    mult = mybir.AluOpType.mult

    vald = nc.dram_tensor(f"vald_scratch_{uid}", [T], F32)
    idxd = nc.dram_tensor(f"idxd_scratch_{uid}", [16, C16], I16)

    with (
        tc.tile_pool(name="const", bufs=1) as cpool,
        tc.tile_pool(name="w1s", bufs=4) as w1pool,
        tc.tile_pool(name="sv", bufs=4) as svpool,
        tc.tile_pool(name="outs", bufs=4) as opool,
    ):
        # ---- resident SBUF tensors ----
        x2 = cpool.tile([P, 2, KO, T], BF16, tag="x2")
        xT = x2[:, 0]
        xlo = x2[:, 1]
        wd = cpool.tile([P, KD, DT, P], BF16, tag="wd")
        cb16 = cpool.tile([P, 2, KO, E], BF16, tag="cb16")
        ghi = cb16[:, 0]
        glo = cb16[:, 1]
        cf = cpool.tile([P, 60], F32, tag="cf")
        gb = cf[:, 0:8]
        sel = cf[:, 8:16]
        iota1 = cf[:, 16:32]
        b1 = cf[:, 32:54]
        bs1 = cf[:, 54:60]
        ws1 = cpool.tile([P, JS, KO, 2 * P], BF16, tag="ws1")
        h_s = cpool.tile([P, JS, T], BF16, tag="h_s")
        h_r = cpool.tile([P, JR, CC], BF16, tag="h_r")
        xg = cpool.tile([P, KO, C], BF16, tag="xg")

        nc.sync.dma_start(cf[:], io["cf32"][:])
        nc.sync.dma_start(cb16[:], io["cb16"][:])
        nc.sync.dma_start(x2[:, :, :, ts(0, 256)], io["x2"][:, :, :, ts(0, 256)])
        for j in range(JS):
            nc.sync.dma_start(ws1[:, j], io["ws1t"][:, j])
        for g in range(1, 8):
            nc.sync.dma_start(x2[:, :, :, ts(g, 256)], io["x2"][:, :, :, ts(g, 256)])

        # ---- gate tiles interleaved with shared-expert up blocks so the PE
        # never idles while x streams in (token-chunked DMAs above) ----
        s_all = cpool.tile([P, MT, E], F32, tag="s_all")
        upsum_cm = tc.tile_pool(name="upsum_s", bufs=2, space="PSUM")
        gpsum_cm = tc.tile_pool(name="gpsum", bufs=2, space="PSUM")
        upsum = upsum_cm.__enter__()
        gpsum = gpsum_cm.__enter__()

        def gate_tile(mt):
            pg = gpsum.tile([P, E], F32, tag="pg")
            # logits = xhi@ghi + xlo@ghi + xhi@glo (fp32 accumulate)
            for k in range(KO):
                nc.tensor.matmul(
                    pg[:], xT[:, k, ts(mt, P)], ghi[:, k], start=(k == 0),
                    stop=False,
                )
            for k in range(KO):
                nc.tensor.matmul(
                    pg[:], xlo[:, k, ts(mt, P)], ghi[:, k], start=False,
                    stop=False,
                )
            for k in range(KO):
                nc.tensor.matmul(
                    pg[:], xT[:, k, ts(mt, P)], glo[:, k], start=False,
                    stop=(k == KO - 1),
                )
            nc.scalar.activation(
                s_all[:, mt], pg[:], mybir.ActivationFunctionType.Sigmoid
            )

        def shared_up_block(t):
            # 256-token block t (8 blocks total)
            for j in range(JS):
                pv = upsum.tile([P, 256], F32, tag="pv")
                pgu = upsum.tile([P, 256], F32, tag="pgu")
                for k in range(KO):
                    nc.tensor.matmul(
                        pv[:], ws1[:, j, k, :P], xT[:, k, ts(t, 256)],
                        start=(k == 0), stop=(k == KO - 1),
                    )
                for k in range(KO):
                    nc.tensor.matmul(
                        pgu[:], ws1[:, j, k, P:], xT[:, k, ts(t, 256)],
                        start=(k == 0), stop=(k == KO - 1),
                    )
                sv = svpool.tile([P, 256], F32, tag="sv")
                bias_v = cf[:, 54 + 2 * j : 55 + 2 * j]
                nc.scalar.activation(
                    sv[:], pv[:], mybir.ActivationFunctionType.Sigmoid,
                    bias=bias_v,
                )
                nc.vector.scalar_tensor_tensor(
                    sv[:], pv[:], bias_v, sv[:], add, mult
                )
                nc.vector.scalar_tensor_tensor(
                    h_s[:, j, ts(t, 256)], pgu[:],
                    cf[:, 55 + 2 * j : 56 + 2 * j], sv[:], add, mult,
                )

        for t in range(4):
            gate_tile(2 * t)
            gate_tile(2 * t + 1)
            shared_up_block(t)
        for mt in range(8, MT):
            gate_tile(mt)
        gpsum_cm.__exit__(None, None, None)

        # ---- top-2 mask + weights ----
        gtmp = cpool.tile([P, MT, E], F32, tag="gtmp")
        gtmp2 = cpool.tile([P, MT, E], F32, tag="gtmp2")
        m1 = cpool.tile([P, MT], F32, tag="m1")
        m2 = cpool.tile([P, MT], F32, tag="m2")
        wq = cpool.tile([P, MT], F32, tag="wq")
        msel = cpool.tile([P, MT], F32, tag="msel")
        val = cpool.tile([P, MT], F32, tag="val")
        # s += gate_bias (broadcast over token tiles)
        nc.vector.tensor_tensor(
            s_all[:], s_all[:], cf[:, None, 0:8].to_broadcast((P, MT, E)), add
        )
        nc.vector.reduce_max(m1[:], s_all[:], axis=mybir.AxisListType.X)
        nc.vector.tensor_tensor(
            gtmp[:], s_all[:], m1[:, :, None].to_broadcast((P, MT, E)),
            mybir.AluOpType.is_equal,
        )
        # s2 = s - BIG * eq1
        nc.vector.scalar_tensor_tensor(
            gtmp2[:], gtmp[:], -BIG, s_all[:], mult, add
        )
        nc.vector.reduce_max(m2[:], gtmp2[:], axis=mybir.AxisListType.X)
        # mask = eq1 + eq2  (gtmp <- mask)
        nc.vector.tensor_tensor(
            gtmp2[:], gtmp2[:], m2[:, :, None].to_broadcast((P, MT, E)),
            mybir.AluOpType.is_equal,
        )
        nc.vector.tensor_tensor(gtmp[:], gtmp[:], gtmp2[:], add)
        # msel = sum(mask * sel): 1 iff my expert is in the top-2
        nc.vector.tensor_tensor(
            gtmp2[:], gtmp[:], cf[:, None, 8:16].to_broadcast((P, MT, E)), mult
        )
        nc.vector.reduce_sum(msel[:], gtmp2[:], axis=mybir.AxisListType.X)
        # wq = SCALE * sum(s * mask * sel) / (m1 + m2)
        nc.vector.tensor_tensor(gtmp[:], gtmp[:], s_all[:], mult)
        nc.vector.tensor_tensor(
            gtmp[:], gtmp[:], cf[:, None, 8:16].to_broadcast((P, MT, E)), mult
        )
        nc.vector.reduce_sum(wq[:], gtmp[:], axis=mybir.AxisListType.X)
        nc.vector.tensor_tensor(m1[:], m1[:], m2[:], add)
        nc.vector.reciprocal(m2[:], m1[:])
        nc.vector.tensor_scalar_mul(m2[:], m2[:], SCALE)
        nc.vector.tensor_tensor(wq[:], wq[:], m2[:], mult)
        nc.gpsimd.dma_start(io["wq_out"][:], wq[:])

        # ---- dispatch: compact my expert's token ids ----
        # val = msel * (t+1) - 1  -> token id if selected else -1
        nc.vector.tensor_tensor(val[:], msel[:], iota1, mult)
        nc.vector.tensor_scalar_add(val[:], val[:], -1.0)
        # fold to token order in DRAM: vald[mt*128+p] = val[p, mt]
        nc.gpsimd.dma_start(vald[:].rearrange("(mt p) -> p mt", p=P), val[:])
        # wrapped load + sentinel block (values T..T+C-1, wrapped)
        vw = cpool.tile([16, T // 16 + C16], F32, tag="vw")
        nc.gpsimd.dma_start(vw[:, : T // 16], vald[:].rearrange("(f q) -> q f", q=16))
        nc.gpsimd.dma_start(vw[:, T // 16 :], io["sent"][:])

        idxf = cpool.tile([16, C16], F32, tag="idxf")
        nfd = cpool.tile([1, 1], U32, tag="nfd")
        nc.gpsimd.sparse_gather(idxf[:], vw[:], num_found=nfd[:])
        nc.gpsimd.dma_start(io["idx_out"][:], idxf[:])

        # clamp sentinels to T-1, cast to int16, replicate to 128 partitions
        idxc = cpool.tile([16, C16], F32, tag="idxc")
        nc.vector.tensor_scalar_min(idxc[:], idxf[:], float(T - 1))
        idx16 = cpool.tile([16, C16], I16, tag="idx16")
        nc.vector.tensor_copy(idx16[:], idxc[:])
        nc.gpsimd.dma_start(idxd[:], idx16[:])
        idx128 = cpool.tile([P, C16], I16, tag="idx128")
        for j in range(8):
            nc.gpsimd.dma_start(idx128[16 * j : 16 * (j + 1), :], idxd[:])

        # gather routed token rows from HBM, transposed into [dsub, ko, c]
        nc.gpsimd.dma_gather(
            xg[:],
            io["x16d"][:],
            idx128[:],
            num_idxs=C,
            num_idxs_reg=C,
            elem_size=D,
            transpose=True,
        )

        # ---- last shared-up blocks (overlap the dispatch chain), then wd ----
        for t in range(4, 8):
            shared_up_block(t)
        for kd in range(KD):
            nc.sync.dma_start(wd[:, kd], io["wd"][:, kd])
        w1tiles = [
            w1pool.tile([P, KO, 2 * P], BF16, name=f"w1tile_{j}", tag="w1tile")
            for j in range(JR)
        ]
        for j in range(3):
            nc.sync.dma_start(w1tiles[j][:], io["w1t"][:, j])

        # ---- shared expert down -> dense [D, T] partial ----
        dpsum_cm = tc.tile_pool(name="dpsum_s", bufs=4, space="PSUM")
        dpsum = dpsum_cm.__enter__()
        for dt in range(DT):
            for t in range(NT):
                pd = dpsum.tile([P, 512], F32, tag="pd")
                for j in range(JS):
                    nc.tensor.matmul(
                        pd[:], wd[:, JR + j, dt], h_s[:, j, ts(t, 512)],
                        start=(j == 0), stop=(j == JS - 1),
                    )
                osb = opool.tile([P, 512], BF16, tag="osb")
                if (dt * NT + t) % 2 == 0:
                    nc.vector.tensor_copy(osb[:], pd[:])
                else:
                    nc.scalar.activation(
                        osb[:], pd[:], mybir.ActivationFunctionType.Copy
                    )
                nc.sync.dma_start(
                    io["shared_out"][ts(dt, P), ts(t, 512)], osb[:]
                )
        dpsum_cm.__exit__(None, None, None)
        upsum_cm.__exit__(None, None, None)

        # ---- routed expert up (compacted tokens) ----
        with tc.tile_pool(name="upsum_r", bufs=4, space="PSUM") as upsum:
            for j in range(JR):
                if j + 3 < JR:
                    nc.sync.dma_start(w1tiles[j + 3][:], io["w1t"][:, j + 3])
                w1tile = w1tiles[j]
                for ct in range(CT):
                    pv = upsum.tile([P, CTS], F32, tag="pv")
                    pgu = upsum.tile([P, CTS], F32, tag="pgu")
                    for k in range(KO):
                        nc.tensor.matmul(
                            pv[:], w1tile[:, k, :P], xg[:, k, ts(ct, CTS)],
                            start=(k == 0), stop=(k == KO - 1),
                        )
                    for k in range(KO):
                        nc.tensor.matmul(
                            pgu[:], w1tile[:, k, P:], xg[:, k, ts(ct, CTS)],
                            start=(k == 0), stop=(k == KO - 1),
                        )
                    sv = svpool.tile([P, CTS], F32, tag="sv")
                    bias_v = cf[:, 32 + 2 * j : 33 + 2 * j]
                    nc.scalar.activation(
                        sv[:], pv[:], mybir.ActivationFunctionType.Sigmoid,
                        bias=bias_v,
                    )
                    nc.vector.scalar_tensor_tensor(
                        sv[:], pv[:], bias_v, sv[:], add, mult
                    )
                    nc.vector.scalar_tensor_tensor(
                        h_r[:, j, ts(ct, CTS)], pgu[:],
                        cf[:, 33 + 2 * j : 34 + 2 * j], sv[:], add, mult,
                    )

        # ---- routed expert down -> compacted [D, C] (no gating/bias) ----
        with tc.tile_pool(name="dpsum_r", bufs=4, space="PSUM") as dpsum:
            for dt in range(DT):
                for ct in range(CT):
                    pd = dpsum.tile([P, CTS], F32, tag="pd")
                    for j in range(JR):
                        nc.tensor.matmul(
                            pd[:], wd[:, j, dt], h_r[:, j, ts(ct, CTS)],
                            start=(j == 0), stop=(j == JR - 1),
                        )
                    osb = opool.tile([P, CTS], BF16, tag="osb")
                    nc.vector.tensor_copy(osb[:], pd[:])
                    nc.sync.dma_start(
                        io["rout_out"][ts(dt, P), ts(ct, CTS)], osb[:]
                    )


def build_nc(reps=1):
    nc = bacc.Bacc(None, target_bir_lowering=False, debug=False)
    io = {
        "x16d": nc.declare_dram_parameter("x16d", [T, D], BF16, isOutput=False),
        "x2": nc.declare_dram_parameter("x2", [P, 2, KO, T], BF16, isOutput=False),
        "cb16": nc.declare_dram_parameter("cb16", [P, 2, KO, E], BF16, isOutput=False),
        "cf32": nc.declare_dram_parameter("cf32", [P, 60], F32, isOutput=False),
        "sent": nc.declare_dram_parameter("sent", [16, C16], F32, isOutput=False),
        "w1t": nc.declare_dram_parameter(
            "w1t", [P, JR, KO, 2 * P], BF16, isOutput=False
        ),
        "ws1t": nc.declare_dram_parameter(
            "ws1t", [P, JS, KO, 2 * P], BF16, isOutput=False
        ),
        "wd": nc.declare_dram_parameter("wd", [P, KD, DT, P], BF16, isOutput=False),
        "shared_out": nc.declare_dram_parameter("shared_out", [D, T], BF16, isOutput=True),
        "rout_out": nc.declare_dram_parameter("rout_out", [D, CC], BF16, isOutput=True),
        "idx_out": nc.declare_dram_parameter("idx_out", [16, C16], F32, isOutput=True),
        "wq_out": nc.declare_dram_parameter("wq_out", [P, MT], F32, isOutput=True),
    }
    with tile.TileContext(nc) as tc:
        for r in range(reps):
            _body(tc, io, uid=r)
    nc.compile()
    return nc


def _shared_slices(core):
    """Global shared pair-tile indices owned by `core` (<= JS of them)."""
    counts = [3, 3, 3, 3, 3, 3, 2, 2]
    start = sum(counts[:core])
    return list(range(start, start + counts[core]))


def prep_inputs(inputs):
    """Full problem inputs -> list of 8 per-core in_maps (numpy arrays)."""
    x = np.asarray(inputs["x"], np.float32)
    gate_w = np.asarray(inputs["gate_w"], np.float32)
    gate_bias = np.asarray(inputs["gate_bias"], np.float32)
    W1 = np.asarray(inputs["W1"], np.float32)
    b1 = np.asarray(inputs["b1"], np.float32)
    W2 = np.asarray(inputs["W2"], np.float32)
    Ws1 = np.asarray(inputs["Ws1"], np.float32)
    bs1 = np.asarray(inputs["bs1"], np.float32)
    Ws2 = np.asarray(inputs["Ws2"], np.float32)

    xf = x.reshape(T, D)
    x16d = np.ascontiguousarray(xf.astype(NP_BF16))
    # xT[p, ko, t] = xf[t, ko*128+p]
    xT32 = np.ascontiguousarray(xf.T.reshape(KO, P, T).transpose(1, 0, 2))
    xT16 = xT32.astype(NP_BF16)
    xlo16 = (xT32 - xT16.astype(np.float32)).astype(NP_BF16)
    x2 = np.ascontiguousarray(np.stack([xT16, xlo16], axis=1))
    gwT = np.ascontiguousarray(gate_w.T.reshape(KO, P, E).transpose(1, 0, 2)).astype(
        np.float32
    )
    gwhi = gwT.astype(NP_BF16)
    gwlo = (gwT - gwhi.astype(np.float32)).astype(NP_BF16)
    cb16 = np.ascontiguousarray(np.stack([gwhi, gwlo], axis=1))
    gb_b = np.broadcast_to(gate_bias[None, :], (P, E)).astype(np.float32).copy()
    # iota1[p, mt] = token_id + 1 = mt*128 + p + 1
    iota1 = (np.arange(T, dtype=np.float32) + 1.0).reshape(MT, P).T.copy()
    # sentinel block, wrapped: value at (q, f) is T + f*16 + q
    sent = (T + np.arange(C, dtype=np.float32).reshape(C16, 16).T).copy()

    # capacity guard: the compiled capacity C must cover the worst expert
    logits = xf @ gate_w.T
    scores = 1.0 / (1.0 + np.exp(-logits)) + gate_bias[None, :]
    top2 = np.argpartition(-scores, TOPK - 1, axis=1)[:, :TOPK]
    counts = np.bincount(top2.ravel(), minlength=E)
    if counts.max() > CC:
        raise RuntimeError(f"expert token count {counts.max()} exceeds capacity {CC}")

    in_maps = []
    for c in range(N_CORES):
        # routed expert weights: W1[c] [2I, D] -> interleaved v/g pair tiles
        A = W1[c].reshape(2, JR, P, KO, P)  # (vg, j, m, ko, p)
        w1t = np.ascontiguousarray(
            A.transpose(4, 1, 3, 0, 2).reshape(P, JR, KO, 2 * P)
        ).astype(NP_BF16)
        b1t = np.ascontiguousarray(
            b1[c].reshape(2, JR, P).transpose(2, 1, 0).reshape(P, 2 * JR)
        ).astype(np.float32)

        # shared expert slice (padded to JS pair tiles)
        sl = _shared_slices(c)
        A_sh = np.zeros((2, JS, P, D), np.float32)
        bs1t_raw = np.zeros((2, JS, P), np.float32)
        Wd_sh = np.zeros((JS, P, D), np.float32)
        for jj, jglob in enumerate(sl):
            rows = slice(jglob * P, (jglob + 1) * P)
            A_sh[0, jj] = Ws1[rows.start : rows.stop]
            A_sh[1, jj] = Ws1[I_SH + rows.start : I_SH + rows.stop]
            bs1t_raw[0, jj] = bs1[rows]
            bs1t_raw[1, jj] = bs1[I_SH + rows.start : I_SH + rows.stop]
            Wd_sh[jj] = Ws2[:, rows].T
        ws1t = np.ascontiguousarray(
            A_sh.reshape(2, JS, P, KO, P).transpose(4, 1, 3, 0, 2).reshape(
                P, JS, KO, 2 * P
            )
        ).astype(NP_BF16)
        bs1t = np.ascontiguousarray(
            bs1t_raw.transpose(2, 1, 0).reshape(P, 2 * JS)
        ).astype(np.float32)

        # down weights: [W2[c].T ; shared slices] -> [128, KD, DT, 128]
        Wd = np.concatenate([W2[c].T, Wd_sh.reshape(JS * P, D)], axis=0)
        wd = np.ascontiguousarray(
            Wd.reshape(KD, P, DT, P).transpose(1, 0, 2, 3)
        ).astype(NP_BF16)

        sel_b = np.zeros((P, E), np.float32)
        sel_b[:, c] = 1.0

        in_maps.append(
            {
                "x16d": x16d,
                "x2": x2,
                "cb16": cb16,
                "cf32": np.ascontiguousarray(
                    np.concatenate([gb_b, sel_b, iota1, b1t, bs1t], axis=1)
                ),
                "sent": sent,
                "w1t": w1t,
                "ws1t": ws1t,
                "wd": wd,
            }
        )
    return in_maps


_NC_CACHE = {}


def get_nc():
    if "nc" not in _NC_CACHE:
        _NC_CACHE["nc"] = build_nc()
    return _NC_CACHE["nc"]


def combine_outputs(inputs, results):
    """Per-core result dicts -> full [B, S, D] float32 output."""
    b2 = np.asarray(inputs["b2"], np.float64)
    bs2 = np.asarray(inputs["bs2"], np.float64)
    acc = np.zeros((D, T), np.float64)
    for c, r in enumerate(results):
        acc += np.asarray(r["shared_out"]).astype(np.float32)
        ids = np.asarray(r["idx_out"], np.float32).T.ravel()[:CC]  # c = f*16+q
        valid = ids < T
        iv = ids[valid].astype(np.int64)
        wq = np.asarray(r["wq_out"], np.float32).T.ravel()  # [T]
        cols = np.asarray(r["rout_out"]).astype(np.float64)[:, valid]
        acc[:, iv] += (cols + b2[c][:, None]) * wq[iv][None, :]
    acc += bs2[:, None]
    return np.ascontiguousarray(acc.T.reshape(B, S, D).astype(np.float32))


def kernel(**inputs):
    nc = get_nc()
    in_maps = prep_inputs(inputs)
    res = run_bass_kernel_spmd(nc, in_maps, core_ids=list(range(N_CORES)))
    return combine_outputs(inputs, res.results)


if __name__ == "__main__":
    # quick self-drive (requires reference.py next to this file)
    import reference

    inputs = {k: np.asarray(v) for k, v in reference.setup_inputs().items()}
    out = kernel(**inputs)
    exp = np.asarray(reference.reference(**inputs))
    err = np.abs(out - exp).max()
    rel = np.abs(out - exp).max() / np.abs(exp).max()
    print("absmax err:", err, "rel:", rel)


# revision 5
# speedup vs baseline: 1.0397x; 1.0397x over previous
"""Kimi-style MoE (8 routed experts top-2 + shared expert) on 8 Trainium2 cores.

Sparse expert-parallel design. Core c owns routed expert c and computes it only
for the tokens actually routed to it (top-2 of 8 => ~T/4 tokens, capacity
C=768), plus a 1/8 intermediate-dim shard of the shared expert.

On-device dispatch per core:
  gate (fp32, replicated) -> top-2 mask -> my-expert mask -> val = token_id|-1
  -> DRAM fold to token order -> [16, T/16] wrapped + sentinel block
  -> gpsimd.sparse_gather (hardware stream compaction) -> compacted token ids
  -> clamp/cast int16 -> replicate to 128 partitions
  -> gpsimd.dma_gather(transpose=True): gathers x rows from HBM transposed
     into the matmul layout [dsub, ko, c].

Each core returns: compacted routed expert outputs [D, C] (no gating applied),
the compacted token ids, the per-token gate weights, and a dense [D, T] shared
partial. The host applies gating + biases and scatter-adds (the baseline
already did its 8-way partial reduce on host).

Expert matmuls run in bf16 (fp32 PSUM); the gate runs in fp32 because top-k
selection is precision-critical.
"""

import sys

for _p in ("/opt/trn_rl_repo", "/opt/pypackages"):
    if _p not in sys.path:
        sys.path.insert(0, _p)

import numpy as np
import ml_dtypes

import concourse.bass as bass
import concourse.mybir as mybir
import concourse.tile as tile
from concourse import bacc
from concourse.bass import ts
from concourse.bass_utils import run_bass_kernel_spmd

BF16 = mybir.dt.bfloat16
F32 = mybir.dt.float32
I16 = mybir.dt.int16
U32 = mybir.dt.uint32
NP_BF16 = ml_dtypes.bfloat16

# Problem shapes (hardcoded per the contract).
B, S, D = 2, 1024, 1024
E, TOPK = 8, 2
I = 1408
N_SHARED = 2
I_SH = N_SHARED * I          # 2816
SCALE = 2.5
T = B * S                    # 2048
P = 128
NT = T // 512                # 4 free-dim tiles of 512 tokens (shared expert)
KO = D // P                  # 8 contraction subtiles
JR = I // P                  # 11 routed (v,g) pair tiles
JS = 3                       # shared pair tiles per core (padded)
KD = JR + JS                 # 14 down-proj contraction tiles
DT = D // P                  # 8 output partition tiles
MT = T // P                  # 16 token tiles in the gate
N_CORES = 8

C = 768                      # routed gather capacity (dma_gather needs %128)
C16 = C // 16                # 48
CC = 704                     # routed compute capacity (>= max expert count)
CT = 2                       # token tiles for routed compute
CTS = CC // CT               # 352

BIG = 1.0e9


def _body(tc, io, uid=0):
    nc = tc.nc
    add = mybir.AluOpType.add
    mult = mybir.AluOpType.mult

    vald = nc.dram_tensor(f"vald_scratch_{uid}", [T], F32)
    idxd = nc.dram_tensor(f"idxd_scratch_{uid}", [16, C16], I16)

    with (
        tc.tile_pool(name="const", bufs=1) as cpool,
        tc.tile_pool(name="w1s", bufs=6) as w1pool,
        tc.tile_pool(name="sv", bufs=4) as svpool,
        tc.tile_pool(name="outs", bufs=4) as opool,
    ):
        # ---- resident SBUF tensors ----
        x2 = cpool.tile([P, 2, KO, T], BF16, tag="x2")
        xT = x2[:, 0]
        xlo = x2[:, 1]
        wd = cpool.tile([P, KD, DT, P], BF16, tag="wd")
        cb16 = cpool.tile([P, 2, KO, E], BF16, tag="cb16")
        ghi = cb16[:, 0]
        glo = cb16[:, 1]
        cf = cpool.tile([P, 60], F32, tag="cf")
        gb = cf[:, 0:8]
        sel = cf[:, 8:16]
        iota1 = cf[:, 16:32]
        b1 = cf[:, 32:54]
        bs1 = cf[:, 54:60]
        ws1 = cpool.tile([P, JS, KO, 2 * P], BF16, tag="ws1")
        h_s = cpool.tile([P, JS, T], BF16, tag="h_s")
        h_r = cpool.tile([P, JR, CC], BF16, tag="h_r")
        xg = cpool.tile([P, KO, C], BF16, tag="xg")

        nc.sync.dma_start(cf[:], io["cf32"][:])
        nc.sync.dma_start(cb16[:], io["cb16"][:])
        nc.sync.dma_start(x2[:, :, :, ts(0, 256)], io["x2"][:, :, :, ts(0, 256)])
        for j in range(JS):
            nc.sync.dma_start(ws1[:, j], io["ws1t"][:, j])
        for g in range(1, 8):
            nc.sync.dma_start(x2[:, :, :, ts(g, 256)], io["x2"][:, :, :, ts(g, 256)])

        # ---- gate tiles interleaved with shared-expert up blocks so the PE
        # never idles while x streams in (token-chunked DMAs above) ----
        s_all = cpool.tile([P, MT, E], F32, tag="s_all")
        upsum_cm = tc.tile_pool(name="upsum_s", bufs=2, space="PSUM")
        gpsum_cm = tc.tile_pool(name="gpsum", bufs=2, space="PSUM")
        upsum = upsum_cm.__enter__()
        gpsum = gpsum_cm.__enter__()

        def gate_tile(mt):
            pg = gpsum.tile([P, E], F32, tag="pg")
            # logits = xhi@ghi + xlo@ghi + xhi@glo (fp32 accumulate)
            for k in range(KO):
                nc.tensor.matmul(
                    pg[:], xT[:, k, ts(mt, P)], ghi[:, k], start=(k == 0),
                    stop=False,
                )
            for k in range(KO):
                nc.tensor.matmul(
                    pg[:], xlo[:, k, ts(mt, P)], ghi[:, k], start=False,
                    stop=False,
                )
            for k in range(KO):
                nc.tensor.matmul(
                    pg[:], xT[:, k, ts(mt, P)], glo[:, k], start=False,
                    stop=(k == KO - 1),
                )
            nc.scalar.activation(
                s_all[:, mt], pg[:], mybir.ActivationFunctionType.Sigmoid
            )

        def shared_up_block(t):
            # 256-token block t (8 blocks total)
            for j in range(JS):
                pv = upsum.tile([P, 256], F32, tag="pv")
                pgu = upsum.tile([P, 256], F32, tag="pgu")
                for k in range(KO):
                    nc.tensor.matmul(
                        pv[:], ws1[:, j, k, :P], xT[:, k, ts(t, 256)],
                        start=(k == 0), stop=(k == KO - 1),
                    )
                for k in range(KO):
                    nc.tensor.matmul(
                        pgu[:], ws1[:, j, k, P:], xT[:, k, ts(t, 256)],
                        start=(k == 0), stop=(k == KO - 1),
                    )
                sv = svpool.tile([P, 256], F32, tag="sv")
                bias_v = cf[:, 54 + 2 * j : 55 + 2 * j]
                nc.scalar.activation(
                    sv[:], pv[:], mybir.ActivationFunctionType.Sigmoid,
                    bias=bias_v,
                )
                nc.vector.scalar_tensor_tensor(
                    sv[:], pv[:], bias_v, sv[:], add, mult
                )
                nc.vector.scalar_tensor_tensor(
                    h_s[:, j, ts(t, 256)], pgu[:],
                    cf[:, 55 + 2 * j : 56 + 2 * j], sv[:], add, mult,
                )

        for t in range(4):
            gate_tile(2 * t)
            gate_tile(2 * t + 1)
            shared_up_block(t)
        for mt in range(8, MT):
            gate_tile(mt)
        gpsum_cm.__exit__(None, None, None)

        # ---- top-2 mask + weights ----
        gtmp = cpool.tile([P, MT, E], F32, tag="gtmp")
        gtmp2 = cpool.tile([P, MT, E], F32, tag="gtmp2")
        m1 = cpool.tile([P, MT], F32, tag="m1")
        m2 = cpool.tile([P, MT], F32, tag="m2")
        wq = cpool.tile([P, MT], F32, tag="wq")
        msel = cpool.tile([P, MT], F32, tag="msel")
        val = cpool.tile([P, MT], F32, tag="val")
        # s += gate_bias (broadcast over token tiles)
        nc.vector.tensor_tensor(
            s_all[:], s_all[:], cf[:, None, 0:8].to_broadcast((P, MT, E)), add
        )
        nc.vector.reduce_max(m1[:], s_all[:], axis=mybir.AxisListType.X)
        nc.vector.tensor_tensor(
            gtmp[:], s_all[:], m1[:, :, None].to_broadcast((P, MT, E)),
            mybir.AluOpType.is_equal,
        )
        # s2 = s - BIG * eq1
        nc.vector.scalar_tensor_tensor(
            gtmp2[:], gtmp[:], -BIG, s_all[:], mult, add
        )
        nc.vector.reduce_max(m2[:], gtmp2[:], axis=mybir.AxisListType.X)
        # mask = eq1 + eq2  (gtmp <- mask)
        nc.vector.tensor_tensor(
            gtmp2[:], gtmp2[:], m2[:, :, None].to_broadcast((P, MT, E)),
            mybir.AluOpType.is_equal,
        )
        nc.vector.tensor_tensor(gtmp[:], gtmp[:], gtmp2[:], add)
        # msel = sum(mask * sel): 1 iff my expert is in the top-2
        nc.vector.tensor_tensor(
            gtmp2[:], gtmp[:], cf[:, None, 8:16].to_broadcast((P, MT, E)), mult
        )
        nc.vector.reduce_sum(msel[:], gtmp2[:], axis=mybir.AxisListType.X)
        # wq = SCALE * sum(s * mask * sel) / (m1 + m2)
        nc.vector.tensor_tensor(gtmp[:], gtmp[:], s_all[:], mult)
        nc.vector.tensor_tensor(
            gtmp[:], gtmp[:], cf[:, None, 8:16].to_broadcast((P, MT, E)), mult
        )
        nc.vector.reduce_sum(wq[:], gtmp[:], axis=mybir.AxisListType.X)
        nc.vector.tensor_tensor(m1[:], m1[:], m2[:], add)
        nc.vector.reciprocal(m2[:], m1[:])
        nc.vector.tensor_scalar_mul(m2[:], m2[:], SCALE)
        nc.vector.tensor_tensor(wq[:], wq[:], m2[:], mult)
        nc.gpsimd.dma_start(io["wq_out"][:], wq[:])

        # ---- dispatch: compact my expert's token ids ----
        # val = msel * (t+1) - 1  -> token id if selected else -1
        nc.vector.tensor_tensor(val[:], msel[:], iota1, mult)
        nc.vector.tensor_scalar_add(val[:], val[:], -1.0)
        # fold to token order in DRAM: vald[mt*128+p] = val[p, mt]
        nc.gpsimd.dma_start(vald[:].rearrange("(mt p) -> p mt", p=P), val[:])
        # wrapped load + sentinel block (values T..T+C-1, wrapped)
        vw = cpool.tile([16, T // 16 + C16], F32, tag="vw")
        nc.gpsimd.dma_start(vw[:, : T // 16], vald[:].rearrange("(f q) -> q f", q=16))
        nc.gpsimd.dma_start(vw[:, T // 16 :], io["sent"][:])

        idxf = cpool.tile([16, C16], F32, tag="idxf")
        nfd = cpool.tile([1, 1], U32, tag="nfd")
        nc.gpsimd.sparse_gather(idxf[:], vw[:], num_found=nfd[:])
        nc.gpsimd.dma_start(io["idx_out"][:], idxf[:])

        # clamp sentinels to T-1, cast to int16, replicate to 128 partitions
        idxc = cpool.tile([16, C16], F32, tag="idxc")
        nc.vector.tensor_scalar_min(idxc[:], idxf[:], float(T - 1))
        idx16 = cpool.tile([16, C16], I16, tag="idx16")
        nc.vector.tensor_copy(idx16[:], idxc[:])
        nc.gpsimd.dma_start(idxd[:], idx16[:])
        idx128 = cpool.tile([P, C16], I16, tag="idx128")
        for j in range(8):
            nc.gpsimd.dma_start(idx128[16 * j : 16 * (j + 1), :], idxd[:])

        # gather routed token rows from HBM, transposed into [dsub, ko, c]
        nc.gpsimd.dma_gather(
            xg[:],
            io["x16d"][:],
            idx128[:],
            num_idxs=C,
            num_idxs_reg=C,
            elem_size=D,
            transpose=True,
        )

        # ---- last shared-up blocks (overlap the dispatch chain), then wd ----
        for t in range(4, 8):
            shared_up_block(t)
        for kd in range(KD):
            nc.sync.dma_start(wd[:, kd], io["wd"][:, kd])
        w1tiles = [
            w1pool.tile([P, KO, 2 * P], BF16, name=f"w1tile_{j}", tag="w1tile")
            for j in range(JR)
        ]
        for j in range(5):
            nc.sync.dma_start(w1tiles[j][:], io["w1t"][:, j])

        # ---- shared expert down -> dense [D, T] partial ----
        dpsum_cm = tc.tile_pool(name="dpsum_s", bufs=4, space="PSUM")
        dpsum = dpsum_cm.__enter__()
        for dt in range(DT):
            for t in range(NT):
                pd = dpsum.tile([P, 512], F32, tag="pd")
                for j in range(JS):
                    nc.tensor.matmul(
                        pd[:], wd[:, JR + j, dt], h_s[:, j, ts(t, 512)],
                        start=(j == 0), stop=(j == JS - 1),
                    )
                osb = opool.tile([P, 512], BF16, tag="osb")
                if (dt * NT + t) % 2 == 0:
                    nc.vector.tensor_copy(osb[:], pd[:])
                else:
                    nc.scalar.activation(
                        osb[:], pd[:], mybir.ActivationFunctionType.Copy
                    )
                nc.sync.dma_start(
                    io["shared_out"][ts(dt, P), ts(t, 512)], osb[:]
                )
        dpsum_cm.__exit__(None, None, None)
        upsum_cm.__exit__(None, None, None)

        # ---- routed expert up (compacted tokens) ----
        with tc.tile_pool(name="upsum_r", bufs=4, space="PSUM") as upsum:
            for j in range(JR):
                if j + 5 < JR:
                    nc.sync.dma_start(w1tiles[j + 5][:], io["w1t"][:, j + 5])
                w1tile = w1tiles[j]
                for ct in range(CT):
                    pv = upsum.tile([P, CTS], F32, tag="pv")
                    pgu = upsum.tile([P, CTS], F32, tag="pgu")
                    for k in range(KO):
                        nc.tensor.matmul(
                            pv[:], w1tile[:, k, :P], xg[:, k, ts(ct, CTS)],
                            start=(k == 0), stop=(k == KO - 1),
                        )
                    for k in range(KO):
                        nc.tensor.matmul(
                            pgu[:], w1tile[:, k, P:], xg[:, k, ts(ct, CTS)],
                            start=(k == 0), stop=(k == KO - 1),
                        )
                    sv = svpool.tile([P, CTS], F32, tag="sv")
                    bias_v = cf[:, 32 + 2 * j : 33 + 2 * j]
                    nc.scalar.activation(
                        sv[:], pv[:], mybir.ActivationFunctionType.Sigmoid,
                        bias=bias_v,
                    )
                    nc.vector.scalar_tensor_tensor(
                        sv[:], pv[:], bias_v, sv[:], add, mult
                    )
                    nc.vector.scalar_tensor_tensor(
                        h_r[:, j, ts(ct, CTS)], pgu[:],
                        cf[:, 33 + 2 * j : 34 + 2 * j], sv[:], add, mult,
                    )

        # ---- routed expert down -> compacted [D, C] (no gating/bias) ----
        with tc.tile_pool(name="dpsum_r", bufs=4, space="PSUM") as dpsum:
            for dt in range(DT):
                for ct in range(CT):
                    pd = dpsum.tile([P, CTS], F32, tag="pd")
                    for j in range(JR):
                        nc.tensor.matmul(
                            pd[:], wd[:, j, dt], h_r[:, j, ts(ct, CTS)],
                            start=(j == 0), stop=(j == JR - 1),
                        )
                    osb = opool.tile([P, CTS], BF16, tag="osb")
                    nc.vector.tensor_copy(osb[:], pd[:])
                    nc.sync.dma_start(
                        io["rout_out"][ts(dt, P), ts(ct, CTS)], osb[:]
                    )


def build_nc(reps=1):
    nc = bacc.Bacc(None, target_bir_lowering=False, debug=False)
    io = {
        "x16d": nc.declare_dram_parameter("x16d", [T, D], BF16, isOutput=False),
        "x2": nc.declare_dram_parameter("x2", [P, 2, KO, T], BF16, isOutput=False),
        "cb16": nc.declare_dram_parameter("cb16", [P, 2, KO, E], BF16, isOutput=False),
        "cf32": nc.declare_dram_parameter("cf32", [P, 60], F32, isOutput=False),
        "sent": nc.declare_dram_parameter("sent", [16, C16], F32, isOutput=False),
        "w1t": nc.declare_dram_parameter(
            "w1t", [P, JR, KO, 2 * P], BF16, isOutput=False
        ),
        "ws1t": nc.declare_dram_parameter(
            "ws1t", [P, JS, KO, 2 * P], BF16, isOutput=False
        ),
        "wd": nc.declare_dram_parameter("wd", [P, KD, DT, P], BF16, isOutput=False),
        "shared_out": nc.declare_dram_parameter("shared_out", [D, T], BF16, isOutput=True),
        "rout_out": nc.declare_dram_parameter("rout_out", [D, CC], BF16, isOutput=True),
        "idx_out": nc.declare_dram_parameter("idx_out", [16, C16], F32, isOutput=True),
        "wq_out": nc.declare_dram_parameter("wq_out", [P, MT], F32, isOutput=True),
    }
    with tile.TileContext(nc) as tc:
        for r in range(reps):
            _body(tc, io, uid=r)
    nc.compile()
    return nc


def _shared_slices(core):
    """Global shared pair-tile indices owned by `core` (<= JS of them)."""
    counts = [3, 3, 3, 3, 3, 3, 2, 2]
    start = sum(counts[:core])
    return list(range(start, start + counts[core]))


def prep_inputs(inputs):
    """Full problem inputs -> list of 8 per-core in_maps (numpy arrays)."""
    x = np.asarray(inputs["x"], np.float32)
    gate_w = np.asarray(inputs["gate_w"], np.float32)
    gate_bias = np.asarray(inputs["gate_bias"], np.float32)
    W1 = np.asarray(inputs["W1"], np.float32)
    b1 = np.asarray(inputs["b1"], np.float32)
    W2 = np.asarray(inputs["W2"], np.float32)
    Ws1 = np.asarray(inputs["Ws1"], np.float32)
    bs1 = np.asarray(inputs["bs1"], np.float32)
    Ws2 = np.asarray(inputs["Ws2"], np.float32)

    xf = x.reshape(T, D)
    x16d = np.ascontiguousarray(xf.astype(NP_BF16))
    # xT[p, ko, t] = xf[t, ko*128+p]
    xT32 = np.ascontiguousarray(xf.T.reshape(KO, P, T).transpose(1, 0, 2))
    xT16 = xT32.astype(NP_BF16)
    xlo16 = (xT32 - xT16.astype(np.float32)).astype(NP_BF16)
    x2 = np.ascontiguousarray(np.stack([xT16, xlo16], axis=1))
    gwT = np.ascontiguousarray(gate_w.T.reshape(KO, P, E).transpose(1, 0, 2)).astype(
        np.float32
    )
    gwhi = gwT.astype(NP_BF16)
    gwlo = (gwT - gwhi.astype(np.float32)).astype(NP_BF16)
    cb16 = np.ascontiguousarray(np.stack([gwhi, gwlo], axis=1))
    gb_b = np.broadcast_to(gate_bias[None, :], (P, E)).astype(np.float32).copy()
    # iota1[p, mt] = token_id + 1 = mt*128 + p + 1
    iota1 = (np.arange(T, dtype=np.float32) + 1.0).reshape(MT, P).T.copy()
    # sentinel block, wrapped: value at (q, f) is T + f*16 + q
    sent = (T + np.arange(C, dtype=np.float32).reshape(C16, 16).T).copy()

    # capacity guard: the compiled capacity C must cover the worst expert
    logits = xf @ gate_w.T
    scores = 1.0 / (1.0 + np.exp(-logits)) + gate_bias[None, :]
    top2 = np.argpartition(-scores, TOPK - 1, axis=1)[:, :TOPK]
    counts = np.bincount(top2.ravel(), minlength=E)
    if counts.max() > CC:
        raise RuntimeError(f"expert token count {counts.max()} exceeds capacity {CC}")

    in_maps = []
    for c in range(N_CORES):
        # routed expert weights: W1[c] [2I, D] -> interleaved v/g pair tiles
        A = W1[c].reshape(2, JR, P, KO, P)  # (vg, j, m, ko, p)
        w1t = np.ascontiguousarray(
            A.transpose(4, 1, 3, 0, 2).reshape(P, JR, KO, 2 * P)
        ).astype(NP_BF16)
        b1t = np.ascontiguousarray(
            b1[c].reshape(2, JR, P).transpose(2, 1, 0).reshape(P, 2 * JR)
        ).astype(np.float32)

        # shared expert slice (padded to JS pair tiles)
        sl = _shared_slices(c)
        A_sh = np.zeros((2, JS, P, D), np.float32)
        bs1t_raw = np.zeros((2, JS, P), np.float32)
        Wd_sh = np.zeros((JS, P, D), np.float32)
        for jj, jglob in enumerate(sl):
            rows = slice(jglob * P, (jglob + 1) * P)
            A_sh[0, jj] = Ws1[rows.start : rows.stop]
            A_sh[1, jj] = Ws1[I_SH + rows.start : I_SH + rows.stop]
            bs1t_raw[0, jj] = bs1[rows]
            bs1t_raw[1, jj] = bs1[I_SH + rows.start : I_SH + rows.stop]
            Wd_sh[jj] = Ws2[:, rows].T
        ws1t = np.ascontiguousarray(
            A_sh.reshape(2, JS, P, KO, P).transpose(4, 1, 3, 0, 2).reshape(
                P, JS, KO, 2 * P
            )
        ).astype(NP_BF16)
        bs1t = np.ascontiguousarray(
            bs1t_raw.transpose(2, 1, 0).reshape(P, 2 * JS)
        ).astype(np.float32)

        # down weights: [W2[c].T ; shared slices] -> [128, KD, DT, 128]
        Wd = np.concatenate([W2[c].T, Wd_sh.reshape(JS * P, D)], axis=0)
        wd = np.ascontiguousarray(
            Wd.reshape(KD, P, DT, P).transpose(1, 0, 2, 3)
        ).astype(NP_BF16)

        sel_b = np.zeros((P, E), np.float32)
        sel_b[:, c] = 1.0

        in_maps.append(
            {
                "x16d": x16d,
                "x2": x2,
                "cb16": cb16,
                "cf32": np.ascontiguousarray(
                    np.concatenate([gb_b, sel_b, iota1, b1t, bs1t], axis=1)
                ),
                "sent": sent,
                "w1t": w1t,
                "ws1t": ws1t,
                "wd": wd,
            }
        )
    return in_maps


_NC_CACHE = {}


def get_nc():
    if "nc" not in _NC_CACHE:
        _NC_CACHE["nc"] = build_nc()
    return _NC_CACHE["nc"]


def combine_outputs(inputs, results):
    """Per-core result dicts -> full [B, S, D] float32 output."""
    b2 = np.asarray(inputs["b2"], np.float64)
    bs2 = np.asarray(inputs["bs2"], np.float64)
    acc = np.zeros((D, T), np.float64)
    for c, r in enumerate(results):
        acc += np.asarray(r["shared_out"]).astype(np.float32)
        ids = np.asarray(r["idx_out"], np.float32).T.ravel()[:CC]  # c = f*16+q
        valid = ids < T
        iv = ids[valid].astype(np.int64)
        wq = np.asarray(r["wq_out"], np.float32).T.ravel()  # [T]
        cols = np.asarray(r["rout_out"]).astype(np.float64)[:, valid]
        acc[:, iv] += (cols + b2[c][:, None]) * wq[iv][None, :]
    acc += bs2[:, None]
    return np.ascontiguousarray(acc.T.reshape(B, S, D).astype(np.float32))


def kernel(**inputs):
    nc = get_nc()
    in_maps = prep_inputs(inputs)
    res = run_bass_kernel_spmd(nc, in_maps, core_ids=list(range(N_CORES)))
    return combine_outputs(inputs, res.results)


if __name__ == "__main__":
    # quick self-drive (requires reference.py next to this file)
    import reference

    inputs = {k: np.asarray(v) for k, v in reference.setup_inputs().items()}
    out = kernel(**inputs)
    exp = np.asarray(reference.reference(**inputs))
    err = np.abs(out - exp).max()
    rel = np.abs(out - exp).max() / np.abs(exp).max()
    print("absmax err:", err, "rel:", rel)
